# revision 13
# baseline (speedup 1.0000x reference)
"""Bass/Tile TRN2 kernel for CrossAttention (B=2, N=4096, D=512, H=8, DH=64).

Sharding: batch*heads over 8 cores — core c handles batch c//4 and heads
(c%4)*2, (c%4)*2+1. Each core computes its two heads' attention and the
partial output projection O_h @ Wo_h; the host sums the 4 partials per batch.

Per-core schedule (software-pipelined so ScalarE (exp) and PE overlap):
  PE emission order per slot t=(ib,jc): QK(t), PV(t-2) — the PV of chunk
  t-2 (which waits on exp(t-2)) never blocks QK(t+1), keeping the
  Act-engine exp stream saturated while PE runs two chunks behind.
  4 of 32 exp tiles per i-block (ib>=1) run on the DVE as a Schraudolph
  fast-exp (PSUM->SBUF copy, then one fused int32(A*x+B) tensor_scalar,
  bitcast to f32) to pull the Activation engine's load just below the
  PE's. Epilogue (denominator reciprocal -> partition transpose ->
  out-proj -> scale/sum/bias) is spread over the next i-block's first
  slots. Q projections are deferred out of i-block 0 (one per prior
  block) to shrink the PE-bound warm-up.
"""

import sys

if "/opt/trn_rl_repo" not in sys.path:
    sys.path.insert(0, "/opt/trn_rl_repo")

import numpy as np

B, N, D = 2, 4096, 512
H, DH = 8, 64
SCALE = DH ** -0.5
P = 128
IB = 512            # i block
NDC = D // P        # 4 contraction chunks for projections
NIB = N // IB       # 8
NJC = N // P        # 32 key chunks per i-block
NQ = IB // P        # 4 out-proj chunks per i-block
NSLOT = NIB * NJC   # 256

# Schraudolph fast-exp on DVE for these jc (i-blocks >= 1)
DVE_JC = (8, 11, 14, 17, 20, 23, 26, 29)
SCH_C = 0.0434
SCH_A = float(2.0 ** 23 * np.log2(np.e) * SCALE)
SCH_B = float((127.0 - SCH_C) * 2.0 ** 23)

_CACHE: dict = {}

import os
K_SCH = os.environ.get("K_SCH", "1") == "1"     # Schraudolph offload
K_DUM = os.environ.get("K_DUM", "1") == "1"     # parity dummy tile
K_PVA = os.environ.get("K_PVA", "1") == "1"     # single-tile V-proj psum
K_PIPE = os.environ.get("K_PIPE", "1") == "1"   # pipelined PV lag


def _build():
    import concourse.mybir as mybir
    from concourse import bacc
    from concourse.tile import TileContext

    f32 = mybir.dt.float32
    f32r = mybir.dt.float32r
    bf16 = mybir.dt.bfloat16
    i32 = mybir.dt.int32
    Exp = mybir.ActivationFunctionType.Exp
    Mult = mybir.AluOpType.mult
    Add = mybir.AluOpType.add

    nc = bacc.Bacc("TRN2")
    xT = nc.dram_tensor("xT", [D, N], bf16, kind="ExternalInput")
    wq = nc.dram_tensor("wq", [D, 2 * DH], bf16, kind="ExternalInput")
    wk = nc.dram_tensor("wk", [D, 2 * DH], bf16, kind="ExternalInput")
    wv = nc.dram_tensor("wv", [D, 2 * DH], bf16, kind="ExternalInput")
    wo = nc.dram_tensor("wo", [2 * DH, D], f32r, kind="ExternalInput")
    bo = nc.dram_tensor("bo", [D], f32, kind="ExternalInput")
    out = nc.dram_tensor("out", [N, D], f32, kind="ExternalOutput")

    with TileContext(nc) as tc, \
         tc.tile_pool(name="persist", bufs=1) as pp:
        xtb = [pp.tile([P, NDC, IB], bf16, name=f"xt{i}", tag=f"xt{i}")
               for i in range(NIB)]
        qtb = [pp.tile([P, IB], f32r, name=f"qt{i}", tag=f"qt{i}")
               for i in range(NIB)]
        ktb = [pp.tile([P, IB], f32r, name=f"kt{i}", tag=f"kt{i}")
               for i in range(NIB)]
        vtb = [pp.tile([P, NQ, 130], f32r, name=f"vt{i}", tag=f"vt{i}")
               for i in range(NIB)]
        wq_sb = pp.tile([P, NDC, 2 * DH], bf16, name="wq_sb", tag="wq")
        wk_sb = pp.tile([P, NDC, 2 * DH], bf16, name="wk_sb", tag="wk")
        # bf16 V-proj runs at full rate with a 128-wide moving operand, so
        # no padding needed (f32r would need free dim >= 256)
        wv_sb = pp.tile([P, NDC, 2 * DH], bf16, name="wv_sb", tag="wv")
        wo_sb = pp.tile([P, D], f32r, name="wo_sb", tag="wo")
        bo_sb = pp.tile([1, D], f32, name="bo_sb", tag="bos")
        bo_bc = pp.tile([P, D], f32, name="bo_bc", tag="bob")
        one_sb = pp.tile([1, 1], f32, name="one_sb", tag="one")

        for dc in range(NDC):
            nc.sync.dma_start(wq_sb[:, dc, :], wq[dc * P:(dc + 1) * P, :])
            nc.sync.dma_start(wk_sb[:, dc, :], wk[dc * P:(dc + 1) * P, :])
            nc.sync.dma_start(wv_sb[:, dc, :], wv[dc * P:(dc + 1) * P, :])
        nc.sync.dma_start(wo_sb[:], wo[:, :])
        nc.sync.dma_start(bo_sb[:], bo[None, :])
        nc.gpsimd.partition_broadcast(bo_bc[:], bo_sb[:])
        nc.vector.memset(one_sb[:], 1.0)
        for ibb in range(NIB):
            nc.vector.memset(vtb[ibb][:, :, 64:65].bitcast(f32), 1.0)
            nc.vector.memset(vtb[ibb][:, :, 129:130].bitcast(f32), 1.0)

        with tc.tile_pool(name="ps", bufs=2, space="PSUM") as ps_pool, \
             tc.tile_pool(name="po", bufs=2, space="PSUM") as po_pool, \
             tc.tile_pool(name="pe", bufs=1, space="PSUM") as pe_pool, \
             tc.tile_pool(name="pt", bufs=7) as pt_pool, \
             tc.tile_pool(name="sc", bufs=2) as sc_pool, \
             tc.tile_pool(name="ep", bufs=3) as ep_pool, \
             tc.tile_pool(name="ot", bufs=3) as ot_pool:

            for ibb in range(NIB):
                for dc in range(NDC):
                    nc.sync.dma_start(xtb[ibb][:, dc, :],
                                      xT[dc * P:(dc + 1) * P,
                                         ibb * IB:(ibb + 1) * IB])

            def emit_kv(b):
                """K (transposed) and V (natural) projections of block b.
                Allocated in the pe ("ep") ring so phase_a never perturbs
                the st double-buffer ring."""
                xt = xtb[b]
                kv = pe_pool.tile([P, 2, IB], f32, tag="ep", name="kv")
                pqk = kv[:, 0, :]
                for dc in range(NDC):
                    nc.tensor.matmul(pqk, wk_sb[:, dc, :], xt[:, dc, :],
                                     start=(dc == 0), stop=(dc == NDC - 1))
                nc.vector.tensor_copy(ktb[b][:], pqk)
                for q in range(NQ):
                    pva = kv[:, 1, q * P:(q + 1) * P]
                    for dc in range(NDC):
                        nc.tensor.matmul(
                            pva, xt[:, dc, q * P:(q + 1) * P],
                            wv_sb[:, dc, :],
                            start=(dc == 0), stop=(dc == NDC - 1))
                for q in range(NQ):
                    nc.vector.tensor_copy(vtb[b][:, q, 0:DH],
                                          kv[:, 1, q * P:q * P + DH])
                    nc.vector.tensor_copy(vtb[b][:, q, 65:65 + DH],
                                          kv[:, 1, q * P + DH:(q + 1) * P])

            def emit_q(b):
                """Q projection (transposed) of block b."""
                xt = xtb[b]
                pq = pe_pool.tile([P, IB], f32, tag="ep", name="pq")
                for dc in range(NDC):
                    nc.tensor.matmul(pq[:], wq_sb[:, dc, :], xt[:, dc, :],
                                     start=(dc == 0), stop=(dc == NDC - 1))
                nc.vector.tensor_copy(qtb[b][:], pq[:])

            # slot state
            st_t = {}      # t -> st PSUM tile
            pt_t = {}      # t -> pt SBUF tile
            o_ib = {}      # ib -> (o0, o1)
            thunks = {}    # slot -> [fn]
            ep_state: dict = {}

            def at(slot, fn):
                thunks.setdefault(slot, []).append(fn)

            def emit_qk(t):
                ib, jc = divmod(t, NJC)
                b, k4 = divmod(jc, NQ)
                k0 = k4 * P
                qt = qtb[ib]
                kt = ktb[b]
                st = ps_pool.tile([P, 2 * IB], f32, tag="st", name="st")
                st_t[t] = st
                nc.tensor.matmul(st[:, 0:IB],
                                 kt[0:DH, k0:k0 + P], qt[0:DH, :],
                                 start=True, stop=True,
                                 tile_position=(0, 0))
                nc.tensor.matmul(st[:, IB:2 * IB],
                                 kt[DH:P, k0:k0 + P], qt[DH:P, :],
                                 start=True, stop=True,
                                 tile_position=(64, 0))

            def emit_exp(t):
                ib, jc = divmod(t, NJC)
                st = st_t[t]
                pt = pt_pool.tile([P, 2 * IB], f32r, tag="pt", name="pt")
                pt_t[t] = pt
                if K_SCH and ib >= 1 and jc in DVE_JC:
                    # Schraudolph fast-exp on DVE. walrus constraints:
                    # tensor_scalar can't read PSUM, int32 out only via an
                    # f32-tile bitcast, and matmul operands must be native
                    # f32r tiles — hence stage copy + ts + f32->f32r copy.
                    stage = sc_pool.tile([P, 2 * IB], f32, tag="sc",
                                         name="sc")
                    nc.vector.tensor_copy(stage[:], st[:])
                    ptf = sc_pool.tile([P, 2 * IB], f32, tag="scf",
                                       name="scf")
                    nc.vector.tensor_scalar(ptf[:].bitcast(i32), stage[:],
                                            SCH_A, SCH_B, Mult, Add)
                    nc.gpsimd.tensor_copy(pt[:], ptf[:])
                else:
                    nc.scalar.activation(pt[:], st[:], Exp, scale=SCALE)

            def emit_pv(t):
                ib, jc = divmod(t, NJC)
                b = jc // NQ
                if jc == 0:
                    o0 = po_pool.tile([65, IB], f32, tag="o", name="o0")
                    o1 = po_pool.tile([65, IB], f32, tag="o", name="o1")
                    o_ib[ib] = (o0, o1)
                o0, o1 = o_ib[ib]
                vt = vtb[b][:, jc % NQ, :]
                pt = pt_t.pop(t)
                nc.tensor.matmul(o0[:], vt[:, 0:65], pt[:, 0:IB],
                                 start=(jc == 0), stop=(jc == NJC - 1))
                nc.tensor.matmul(o1[:], vt[:, 65:130], pt[:, IB:2 * IB],
                                 start=(jc == 0), stop=(jc == NJC - 1))
                del st_t[t]

            def sched_epilogue(ib, t0):
                """Epilogue of i-block ib, spread over slots t0+1..t0+6."""
                def recips():
                    # order frees o0 asap: recip0, ots0, recip1, ots1
                    o0, o1 = o_ib[ib]
                    rinv0 = ep_pool.tile([1, IB], f32, tag="rinv", name="ri0")
                    rinv1 = ep_pool.tile([1, IB], f32, tag="rinv", name="ri1")
                    ots = ep_pool.tile([P, IB], f32r, tag="otn", name="ots")
                    nc.vector.reciprocal(rinv0[:], o0[64:65, :])
                    nc.vector.tensor_copy(ots[0:DH, :], o0[0:DH, :])
                    nc.vector.reciprocal(rinv1[:], o1[64:65, :])
                    nc.vector.tensor_copy(ots[DH:P, :], o1[0:DH, :])
                    ep_state["rinv"] = (rinv0, rinv1)
                    ep_state["ots"] = ots
                    del o_ib[ib]

                def trans():
                    rinv0, rinv1 = ep_state.pop("rinv")
                    rtp = pe_pool.tile([P, 2 * NQ], f32, tag="ep", name="rtp")
                    for q in range(NQ):
                        nc.tensor.matmul(rtp[:, q:q + 1],
                                         rinv0[0:1, q * P:(q + 1) * P],
                                         one_sb[:], start=True, stop=True)
                        nc.tensor.matmul(rtp[:, NQ + q:NQ + q + 1],
                                         rinv1[0:1, q * P:(q + 1) * P],
                                         one_sb[:], start=True, stop=True)
                    rts = ep_pool.tile([P, 2 * NQ], f32, tag="rts", name="rts")
                    nc.vector.tensor_copy(rts[:], rtp[:])
                    ep_state["ro"] = (rts, ep_state.pop("ots"))

                def outq(q):
                    rts, ots = ep_state["ro"]
                    q0, q1 = q * P, (q + 1) * P
                    ppx = pe_pool.tile([P, 2 * D], f32, tag="ep", name="ppx")
                    nc.tensor.matmul(ppx[:, 0:D], ots[0:DH, q0:q1],
                                     wo_sb[0:DH, :], start=True, stop=True,
                                     tile_position=(0, 0))
                    nc.tensor.matmul(ppx[:, D:2 * D], ots[DH:P, q0:q1],
                                     wo_sb[DH:P, :], start=True, stop=True,
                                     tile_position=(64, 0))
                    t0_ = ot_pool.tile([P, D], f32, tag="t0", name="t0")
                    t1_ = ot_pool.tile([P, D], f32, tag="t1", name="t1")
                    nc.vector.tensor_scalar_mul(t0_[:], ppx[:, 0:D],
                                                rts[:, q:q + 1])
                    nc.vector.tensor_scalar_mul(t1_[:], ppx[:, D:2 * D],
                                                rts[:, NQ + q:NQ + q + 1])
                    otile = ot_pool.tile([P, D], f32, tag="out", name="otile")
                    nc.gpsimd.tensor_add(otile[:], t0_[:], t1_[:])
                    nc.gpsimd.tensor_add(otile[:], otile[:], bo_bc[:])
                    nc.sync.dma_start(out[ib * IB + q0:ib * IB + q1, :],
                                      otile[:])

                at(t0 + 1, recips)
                at(t0 + 2, trans)
                for q in range(NQ):
                    at(t0 + 3 + q, lambda q=q: outq(q))

            # Q projections: block 0 upfront, block b at mid prior i-block
            emit_q(0)
            for b in range(1, NIB):
                at((b - 1) * NJC + 16, lambda b=b: emit_q(b))
            # epilogues
            for ib in range(NIB):
                sched_epilogue(ib, (ib + 1) * NJC)

            # PV emission schedule: lag 2, boundary drain + catch-up
            import collections
            pend = collections.deque()

            def pops_for(t):
                if not K_PIPE:
                    return 1
                ph = t % NJC
                if ph == 0:
                    # drain all PVs of the previous i-block
                    t0 = t
                    return sum(1 for i in pend if i < t0)
                if ph in (1, 2, 3):
                    return 0
                if ph in (4, 5):
                    return 2
                return 1

            SCH_LAG = 4

            def pop_one(t):
                # prefer the oldest tile whose exp chain is surely done:
                # sch tiles need >= SCH_LAG slots for the 3-op DVE chain
                for idx in range(len(pend)):
                    i = pend[idx]
                    ib_i, jc_i = divmod(i, NJC)
                    is_sch = K_SCH and ib_i >= 1 and jc_i in DVE_JC
                    if is_sch and (t - i) < SCH_LAG and t % NJC != 0:
                        continue
                    del pend[idx]
                    return emit_pv(i)
                return emit_pv(pend.popleft())

            emitted = set()

            for t in range(NSLOT):
                for fn in thunks.pop(t, ()):
                    fn()
                if t < NJC and t % NQ == 0:
                    emit_kv(t // NQ)
                if t not in emitted:
                    emit_qk(t)
                    emit_exp(t)
                    pend.append(t)
                    emitted.add(t)
                if t % NJC == 0 and t + 1 < NSLOT:
                    # pre-emit next slot's QK so the boundary PV drain
                    # doesn't starve the Act engine
                    emit_qk(t + 1)
                    emit_exp(t + 1)
                    pend.append(t + 1)
                    emitted.add(t + 1)
                for _ in range(pops_for(t)):
                    pop_one(t)
            while pend:
                emit_pv(pend.popleft())
            for s in sorted(thunks):
                for fn in thunks[s]:
                    fn()
            thunks.clear()

    nc.compile()
    return nc


def _get_nc():
    if "nc" not in _CACHE:
        _CACHE["nc"] = _build()
    return _CACHE["nc"]


def kernel(x, Wq, Wk, Wv, Wo, bo):
    import ml_dtypes
    from concourse.bass_utils import run_bass_kernel_spmd

    bf = ml_dtypes.bfloat16

    x = np.asarray(x, dtype=np.float32)
    Wq = np.asarray(Wq, dtype=np.float32)
    Wk = np.asarray(Wk, dtype=np.float32)
    Wv = np.asarray(Wv, dtype=np.float32)
    Wo = np.asarray(Wo, dtype=np.float32)
    bo = np.asarray(bo, dtype=np.float32)

    nc = _get_nc()

    xTs = [np.ascontiguousarray(x[b].T).astype(bf) for b in range(B)]
    zeros_bo = np.zeros_like(bo)
    in_maps = []
    for c in range(8):
        b, p = c // 4, c % 4
        sl = slice(p * 2 * DH, (p + 1) * 2 * DH)
        in_maps.append({
            "xT": xTs[b],
            "wq": np.ascontiguousarray(Wq[:, sl]).astype(bf),
            "wk": np.ascontiguousarray(Wk[:, sl]).astype(bf),
            "wv": np.ascontiguousarray(Wv[:, sl]).astype(bf),
            "wo": np.ascontiguousarray(Wo[sl, :]),
            "bo": bo if p == 0 else zeros_bo,
        })

    try:
        res = run_bass_kernel_spmd(nc, in_maps, core_ids=list(range(8)))
    except Exception:
        # transient device wedge (NRT_EXEC_UNIT_UNRECOVERABLE) — retry once
        import time as _time
        _time.sleep(45)
        res = run_bass_kernel_spmd(nc, in_maps, core_ids=list(range(8)))
    parts = [res.results[c]["out"] for c in range(8)]
    full = np.stack([
        parts[0] + parts[1] + parts[2] + parts[3],
        parts[4] + parts[5] + parts[6] + parts[7],
    ]).astype(np.float32)
    return full
